# revision 1
# baseline (speedup 1.0000x reference)
"""BotGAT Trainium2 kernel: 8-core node-sharded GAT with per-edge indirect gathers.

Per core:
  Node phase (T-layout, features on partitions):
    xT = lrelu(concat(W_m.T @ modT) + b_enc)      [128, NPC]
    uT = lrelu(W_in.T @ xT + b_in)                [128, NPC]
    hT = gat_W.T @ srcT                           [128, NPC]
    table rows (node-major, interleaved): [(h_head(C)|1.0) x H | als(4) | pad] bf16
    ald rows [NPC, 4] f32; AllGather local tables -> full table in DRAM.
  Edge phase (edges packed by dst tile, MB batches of 128 per tile):
    G   = indirect row gather of table[src]       [128, BC, ROW]
    ALD = indirect row gather of ald[dst_local]   [128, BC, 4]
    ex = exp(lrelu(als + ald, 0.2)) per chunk
    per tile: M_all = (iota == dstloc); G_w = G * ex-bcast   (one DVE op each)
    per batch: psum[s, :] += M.T @ G_w   (den accumulates in the "1" cols)
    per tile: x[s, h, c] = psum_h/den_h; then transpose sweep -> xT (+bias)
"""
import sys
sys.path.insert(0, "/opt/trn_rl_repo")
import numpy as np
from dataclasses import dataclass
from contextlib import ExitStack

from concourse import bass, bacc, mybir, tile

F32 = mybir.dt.float32
BF16 = mybir.dt.bfloat16
I32 = mybir.dt.int32
P = 128


@dataclass(frozen=True)
class Cfg:
    n_real: int
    ncores: int
    tpc: int             # tiles per core
    mb: int              # batches (of 128 edges) per tile
    ct: int              # tiles per gather chunk, divides tpc
    table_dt: object
    vec_off: bool = False  # use multi-offset chunked gathers (needs vector_dynamic_offsets)

    @property
    def npc(self): return self.tpc * P
    @property
    def npad(self): return self.ncores * self.npc
    @property
    def nbatch(self): return self.tpc * self.mb
    @property
    def bc(self): return self.ct * self.mb
    @property
    def nch(self): return self.tpc // self.ct
    @property
    def eslots(self): return self.nbatch * P


FULL = Cfg(n_real=100000, ncores=8, tpc=98, mb=18, ct=2, table_dt=BF16)

ROW = 136  # L1: 4*(32+1)+4 = 136 ; L2: 1*(128+1)+4+3pad = 136


def _segs(n, maxseg=512):
    out = []
    while n > 0:
        s = min(maxseg, n)
        out.append(s); n -= s
    return out


# ---------------------------------------------------------------------------
def build(cfg: Cfg):
    nc = bacc.Bacc("TRN2", target_bir_lowering=False, debug=False,
                   num_devices=cfg.ncores)
    NPC = cfg.npc
    TDT = cfg.table_dt
    segs = _segs(NPC)

    inp = {}
    def di(name, shape, dt):
        inp[name] = nc.dram_tensor(name, list(shape), dt, kind="ExternalInput")

    di("desT", [768, NPC], F32); di("tweetT", [768, NPC], F32)
    di("numT", [5, NPC], F32);   di("catT", [3, NPC], F32)
    di("W_des", [768, 32], F32); di("W_tw", [768, 32], F32)
    di("W_np", [5, 32], F32);    di("W_cp", [3, 32], F32)
    di("b_enc", [128, 1], F32)
    di("W_in", [128, 128], F32); di("b_in", [128, 1], F32)
    di("gat1_W", [128, 128], F32); di("A_src1", [128, 4], F32); di("A_dst1", [128, 4], F32)
    di("b1", [128, 1], F32)
    di("gat2_W", [128, 128], F32); di("A_src2", [128, 1], F32); di("A_dst2", [128, 1], F32)
    di("b2", [128, 1], F32)
    di("W_o1", [128, 128], F32); di("b_o1", [128, 1], F32)
    di("W_o2", [128, 2], F32);   di("b_o2", [2, 1], F32)
    di("iota_bf", [P, P], BF16)
    di("ident", [P, P], F32)
    di("ident_bf", [P, P], BF16)
    di("idx_src", [P, cfg.nbatch], I32)
    di("dstloc", [P, cfg.nbatch], BF16)

    outp = nc.dram_tensor("outT", [2, NPC], F32, kind="ExternalOutput")

    tb_loc = [nc.dram_tensor(f"tb_loc{l}", [NPC, ROW], TDT) for l in (1, 2)]
    table = [nc.dram_tensor(f"table{l}", [cfg.npad, ROW], TDT) for l in (1, 2)]
    ald = [nc.dram_tensor(f"ald{l}", [NPC, 4], F32) for l in (1, 2)]

    with tile.TileContext(nc) as tc:
      with ExitStack() as top:
        consts = top.enter_context(tc.tile_pool(name="consts", bufs=1))
        slabs = top.enter_context(tc.tile_pool(name="slabs", bufs=2))

        def big():
            return slabs.tile([128, NPC], F32, tag="big", name="bigslab")

        iota_t = consts.tile([P, P], BF16, tag="iota")
        nc.sync.dma_start(out=iota_t[:], in_=inp["iota_bf"][:, :])
        ident_t = consts.tile([P, P], F32, tag="ident")
        nc.sync.dma_start(out=ident_t[:], in_=inp["ident"][:, :])
        identb_t = consts.tile([P, P], BF16, tag="identb")
        nc.sync.dma_start(out=identb_t[:], in_=inp["ident_bf"][:, :])
        smallw = {}
        for name, shape in [("W_in", (128, 128)), ("gat1_W", (128, 128)),
                            ("gat2_W", (128, 128)), ("W_o1", (128, 128)),
                            ("W_o2", (128, 2)), ("A_src1", (128, 4)),
                            ("A_dst1", (128, 4)), ("A_src2", (128, 1)),
                            ("A_dst2", (128, 1)), ("b_enc", (128, 1)),
                            ("b_in", (128, 1)), ("b1", (128, 1)),
                            ("b2", (128, 1)), ("b_o1", (128, 1)), ("b_o2", (2, 1))]:
            t = consts.tile(list(shape), F32, tag=f"c_{name}")
            nc.sync.dma_start(out=t[:], in_=inp[name][:, :])
            smallw[name] = t
        dstloc_t = consts.tile([P, cfg.nbatch], BF16, tag="dstloc")
        nc.sync.dma_start(out=dstloc_t[:], in_=inp["dstloc"][:, :])

        # =================== encoders -> xT ===================
        xT = big()
        with ExitStack() as ph:
            lp = ph.enter_context(tc.tile_pool(name="enc_load", bufs=3))
            wp = ph.enter_context(tc.tile_pool(name="enc_w", bufs=1))
            pp = ph.enter_context(tc.tile_pool(name="enc_psum", bufs=4, space="PSUM"))

            mods = [("desT", "W_des", 768, 0), ("tweetT", "W_tw", 768, 32),
                    ("numT", "W_np", 5, 64), ("catT", "W_cp", 3, 96)]
            wts = {}
            for mod, wn, K, _ in mods:
                nk = (K + 127) // 128
                w = wp.tile([min(K, 128), nk * 32], F32, tag=f"w_{wn}")
                for kc in range(nk):
                    k0, k1 = kc * 128, min(K, (kc + 1) * 128)
                    nc.sync.dma_start(out=w[0:k1 - k0, kc * 32:(kc + 1) * 32],
                                      in_=inp[wn][k0:k1, :])
                wts[mod] = (w, nk, K)

            off = 0
            for seg in segs:
                for mod, wn, K, pbase in mods:
                    w, nk, K = wts[mod]
                    ps = pp.tile([32, 512], F32, space="PSUM", tag="enc_ps")
                    for kc in range(nk):
                        k0, k1 = kc * 128, min(K, (kc + 1) * 128)
                        rt = lp.tile([128, 512], F32, tag="enc_rhs")
                        nc.sync.dma_start(out=rt[0:k1 - k0, 0:seg],
                                          in_=inp[mod][k0:k1, off:off + seg])
                        nc.tensor.matmul(out=ps[:, 0:seg],
                                         lhsT=w[0:k1 - k0, kc * 32:(kc + 1) * 32],
                                         rhs=rt[0:k1 - k0, 0:seg],
                                         start=(kc == 0), stop=(kc == nk - 1))
                    nc.scalar.activation(out=xT[pbase:pbase + 32, off:off + seg],
                                         in_=ps[:, 0:seg],
                                         func=mybir.ActivationFunctionType.Lrelu,
                                         bias=smallw["b_enc"][pbase:pbase + 32, :],
                                         alpha=0.01)
                off += seg

        # =================== uT ===================
        uT = big()
        with ExitStack() as ph:
            pp = ph.enter_context(tc.tile_pool(name="nd_psum", bufs=4, space="PSUM"))
            off = 0
            for seg in segs:
                ps = pp.tile([128, 512], F32, space="PSUM", tag="nd_ps")
                nc.tensor.matmul(out=ps[:, 0:seg], lhsT=smallw["W_in"][:, :],
                                 rhs=xT[:, off:off + seg], start=True, stop=True)
                nc.scalar.activation(out=uT[:, off:off + seg], in_=ps[:, 0:seg],
                                     func=mybir.ActivationFunctionType.Lrelu,
                                     bias=smallw["b_in"][:, :], alpha=0.01)
                off += seg

        # ============ per-layer table build + edge phase ===============
        def build_table(srcT, gwn, asn, adn, H, lix):
            C = 128 // H
            gw, asrc, adst = smallw[gwn], smallw[asn], smallw[adn]
            hT = big()
            with ExitStack() as ph:
                pp = ph.enter_context(tc.tile_pool(name="tb_psum", bufs=4, space="PSUM"))
                off = 0
                for seg in segs:
                    ps = pp.tile([128, 512], F32, space="PSUM", tag="tb_ps")
                    nc.tensor.matmul(out=ps[:, 0:seg], lhsT=gw[:, :],
                                     rhs=srcT[:, off:off + seg], start=True, stop=True)
                    nc.scalar.copy(out=hT[:, off:off + seg], in_=ps[:, 0:seg])
                    off += seg

            GB = 8
            with ExitStack() as ph:
                ap_ = ph.enter_context(tc.tile_pool(name="asm", bufs=2))
                pp = ph.enter_context(tc.tile_pool(name="asm_psum", bufs=2, space="PSUM"))
                base = H * (C + 1)
                for g0 in range(0, cfg.tpc, GB):
                    gn = min(GB, cfg.tpc - g0)
                    asm = ap_.tile([P, GB * ROW], TDT, tag="asm_t")
                    asma = ap_.tile([P, GB * 4], F32, tag="asm_a")
                    for j in range(gn):
                        n0 = (g0 + j) * P
                        hblk = hT[:, n0:n0 + P]
                        ph_ = pp.tile([P, P], F32, space="PSUM", tag="asm_ph")
                        nc.tensor.transpose(out=ph_[:], in_=hblk, identity=ident_t[:])
                        dst_h = asm[:, j * ROW:j * ROW + base].rearrange(
                            "p (h c) -> p h c", c=C + 1)
                        nc.scalar.copy(out=dst_h[:, :, 0:C],
                                       in_=ph_[:].rearrange("p (h c) -> p h c", c=C))
                        nc.vector.memset(dst_h[:, :, C:C + 1], 1.0)
                        # als via node-major matmul: [128f,128n].T @ A -> [n, H]
                        pa = pp.tile([P, 4], F32, space="PSUM", tag="asm_pa")
                        nc.tensor.matmul(out=pa[:, 0:H], lhsT=hblk,
                                         rhs=asrc[:, 0:H], start=True, stop=True)
                        nc.scalar.copy(out=asm[:, j * ROW + base:j * ROW + base + H],
                                       in_=pa[:, 0:H])
                        if base + H < ROW:
                            nc.vector.memset(asm[:, j * ROW + base + H:(j + 1) * ROW], 0.0)
                        pd = pp.tile([P, 4], F32, space="PSUM", tag="asm_pd")
                        nc.tensor.matmul(out=pd[:, 0:H], lhsT=hblk,
                                         rhs=adst[:, 0:H], start=True, stop=True)
                        nc.vector.tensor_copy(out=asma[:, j * 4:j * 4 + H],
                                              in_=pd[:, 0:H])
                        if H < 4:
                            nc.vector.memset(asma[:, j * 4 + H:(j + 1) * 4], 0.0)
                    dv = tb_loc[lix][g0 * P:(g0 + gn) * P, :].rearrange(
                        "(b p) c -> p b c", p=P)
                    nc.sync.dma_start(out=dv, in_=asm[:, 0:gn * ROW].rearrange(
                        "p (b c) -> p b c", b=gn))
                    da = ald[lix][g0 * P:(g0 + gn) * P, :].rearrange(
                        "(b p) c -> p b c", p=P)
                    nc.sync.dma_start(out=da, in_=asma[:, 0:gn * 4].rearrange(
                        "p (b c) -> p b c", b=gn))

            nc.gpsimd.collective_compute(
                "AllGather", mybir.AluOpType.bypass,
                replica_groups=[list(range(cfg.ncores))],
                ins=[tb_loc[lix][:, :].opt()],
                outs=[table[lix][:, :].opt()],
            )

        def edge_phase(H, biasn, lix):
            C = 128 // H
            base = H * (C + 1)
            BC = cfg.bc
            x_sb = big()
            with ExitStack() as ph:
                ip_ = ph.enter_context(tc.tile_pool(name="ichunk", bufs=2))
                gp = ph.enter_context(tc.tile_pool(name="gchunk", bufs=2))
                ep = ph.enter_context(tc.tile_pool(name="extiles", bufs=2))
                mp = ph.enter_context(tc.tile_pool(name="mtiles", bufs=2))
                m2p = ph.enter_context(tc.tile_pool(name="m2tiles", bufs=3))
                gwp = ph.enter_context(tc.tile_pool(name="gwtiles", bufs=2))
                pp = ph.enter_context(tc.tile_pool(name="acc_psum", bufs=2, space="PSUM"))
                tp2 = ph.enter_context(tc.tile_pool(name="tr2_psum", bufs=3, space="PSUM"))
                ap2 = ph.enter_context(tc.tile_pool(name="ald_psum", bufs=2, space="PSUM"))
                sp = ph.enter_context(tc.tile_pool(name="small", bufs=4))
                al_ = ph.enter_context(tc.tile_pool(name="aldsb", bufs=1))

                # whole local ald table resident: [128 s, tpc, 4] bf16
                aldsb = al_.tile([P, cfg.tpc * 4], BF16, tag="aldsb", name="aldsb")
                nc.gpsimd.dma_start(
                    out=aldsb[:].rearrange("p (t c) -> p t c", t=cfg.tpc),
                    in_=ald[lix][:, :].rearrange("(t p) c -> p t c", p=P))

                for g in range(cfg.nch):
                    b0 = g * BC
                    isrc = ip_.tile([P, BC], I32, tag="isrc")
                    nc.sync.dma_start(out=isrc[:], in_=inp["idx_src"][:, b0:b0 + BC])

                    G = gp.tile([P, BC * ROW], TDT, tag="G")
                    Gv = G[:].rearrange("p (b r) -> p b r", b=BC)
                    if cfg.vec_off:
                        nc.gpsimd.indirect_dma_start(
                            out=Gv, out_offset=None, in_=table[lix][:, :],
                            in_offset=bass.IndirectOffsetOnAxis(ap=isrc[:, :], axis=0))
                    else:
                        for j in range(BC):
                            nc.gpsimd.indirect_dma_start(
                                out=G[:, j * ROW:(j + 1) * ROW], out_offset=None,
                                in_=table[lix][:, :],
                                in_offset=bass.IndirectOffsetOnAxis(
                                    ap=isrc[:, j:j + 1], axis=0))

                    for ti in range(cfg.ct):
                        t = g * cfg.ct + ti
                        bt = b0 + ti * cfg.mb
                        M = mp.tile([P, cfg.mb * P], BF16, tag="M")
                        nc.vector.tensor_tensor(
                            out=M[:].rearrange("p (b s) -> p b s", b=cfg.mb),
                            in0=iota_t[:].unsqueeze(1).to_broadcast([P, cfg.mb, P]),
                            in1=dstloc_t[:, bt:bt + cfg.mb].unsqueeze(2).to_broadcast([P, cfg.mb, P]),
                            op=mybir.AluOpType.is_equal)
                        # ald per edge: transpose M_b then matmul with ald rows
                        aldp = ap2.tile([P, cfg.mb * 4], F32, space="PSUM", tag="aldp")
                        for lb in range(cfg.mb):
                            m2ps = tp2.tile([P, P], BF16, space="PSUM", tag="m2ps")
                            nc.tensor.transpose(out=m2ps[:], in_=M[:, lb * P:(lb + 1) * P],
                                                identity=identb_t[:])
                            m2sb = m2p.tile([P, P], BF16, tag="m2sb")
                            nc.vector.tensor_copy(out=m2sb[:], in_=m2ps[:])
                            nc.tensor.matmul(
                                out=aldp[:, lb * 4:lb * 4 + H],
                                lhsT=m2sb[:],
                                rhs=aldsb[:].rearrange("p (t c) -> p t c", t=cfg.tpc)[:, t, 0:H],
                                start=True, stop=True)
                        # e = als + ald ; ex = exp(lrelu(e, 0.2)) per tile
                        ef = ep.tile([P, cfg.mb * 4], F32, tag="ef")
                        nc.vector.tensor_tensor(
                            out=ef[:].rearrange("p (b r) -> p b r", b=cfg.mb)[:, :, 0:H],
                            in0=Gv[:, ti * cfg.mb:(ti + 1) * cfg.mb, base:base + H],
                            in1=aldp[:, :].rearrange("p (b r) -> p b r", b=cfg.mb)[:, :, 0:H],
                            op=mybir.AluOpType.add)
                        efv = ef[:].rearrange("p (b r) -> p b r", b=cfg.mb)[:, :, 0:H]
                        nc.scalar.activation(out=efv, in_=efv,
                                             func=mybir.ActivationFunctionType.Lrelu,
                                             alpha=0.2)
                        nc.scalar.activation(out=efv, in_=efv,
                                             func=mybir.ActivationFunctionType.Exp)
                        exb = ep.tile([P, cfg.mb * 4], BF16, tag="exb")
                        nc.vector.tensor_copy(
                            out=exb[:].rearrange("p (b r) -> p b r", b=cfg.mb)[:, :, 0:H],
                            in_=efv)
                        exv = exb[:].rearrange("p (b r) -> p b r", b=cfg.mb)
                        GW = gwp.tile([P, cfg.mb * base], TDT, tag="GW")
                        nc.vector.tensor_tensor(
                            out=GW[:].rearrange("p (b h c) -> p b h c",
                                                b=cfg.mb, h=H),
                            in0=Gv[:, ti * cfg.mb:(ti + 1) * cfg.mb, 0:base].rearrange(
                                "p b (h c) -> p b h c", h=H),
                            in1=exv[:, :, 0:H].unsqueeze(3).to_broadcast([P, cfg.mb, H, C + 1]),
                            op=mybir.AluOpType.mult)
                        ps = pp.tile([P, base], F32, space="PSUM", tag="acc")
                        for lb in range(cfg.mb):
                            nc.tensor.matmul(
                                out=ps[:, :],
                                lhsT=M[:, lb * P:(lb + 1) * P],
                                rhs=GW[:, lb * base:(lb + 1) * base],
                                start=(lb == 0), stop=(lb == cfg.mb - 1))
                        psv = ps[:, :].rearrange("p (h c) -> p h c", h=H)
                        rden = sp.tile([P, 4], F32, tag="rden")
                        nc.vector.tensor_scalar_add(out=rden[:, 0:H],
                                                    in0=psv[:, :, C], scalar1=1e-16)
                        nc.vector.reciprocal(out=rden[:, 0:H], in_=rden[:, 0:H])
                        nc.vector.tensor_tensor(
                            out=x_sb[:, t * P:(t + 1) * P].rearrange(
                                "p (h c) -> p h c", h=H),
                            in0=psv[:, :, 0:C],
                            in1=rden[:, 0:H].unsqueeze(2).to_broadcast([P, H, C]),
                            op=mybir.AluOpType.mult)

            xoutT = big()
            with ExitStack() as ph:
                pp = ph.enter_context(tc.tile_pool(name="tr_psum", bufs=4, space="PSUM"))
                for t in range(cfg.tpc):
                    pt = pp.tile([P, P], F32, space="PSUM", tag="tr_ps")
                    nc.tensor.transpose(out=pt[:], in_=x_sb[:, t * P:(t + 1) * P],
                                        identity=ident_t[:])
                    nc.scalar.activation(out=xoutT[:, t * P:(t + 1) * P], in_=pt[:],
                                         func=mybir.ActivationFunctionType.Identity,
                                         bias=smallw[biasn][:, :])
            return xoutT

        build_table(uT, "gat1_W", "A_src1", "A_dst1", 4, 0)
        x1T = edge_phase(4, "b1", 0)
        build_table(x1T, "gat2_W", "A_src2", "A_dst2", 1, 1)
        x2T = edge_phase(1, "b2", 1)

        # head
        o1T = big()
        with ExitStack() as ph:
            pp = ph.enter_context(tc.tile_pool(name="hd_psum", bufs=4, space="PSUM"))
            op_ = ph.enter_context(tc.tile_pool(name="hd_out", bufs=2))
            off = 0
            for seg in segs:
                ps = pp.tile([128, 512], F32, space="PSUM", tag="hd_ps")
                nc.tensor.matmul(out=ps[:, 0:seg], lhsT=smallw["W_o1"][:, :],
                                 rhs=x2T[:, off:off + seg], start=True, stop=True)
                nc.scalar.activation(out=o1T[:, off:off + seg], in_=ps[:, 0:seg],
                                     func=mybir.ActivationFunctionType.Lrelu,
                                     bias=smallw["b_o1"][:, :], alpha=0.01)
                off += seg
            off = 0
            for seg in segs:
                ps2 = pp.tile([2, 512], F32, space="PSUM", tag="hd_ps2")
                nc.tensor.matmul(out=ps2[:, 0:seg], lhsT=smallw["W_o2"][:, :],
                                 rhs=o1T[:, off:off + seg], start=True, stop=True)
                ot = op_.tile([2, 512], F32, tag="hd_ot")
                nc.scalar.activation(out=ot[:, 0:seg], in_=ps2[:, 0:seg],
                                     func=mybir.ActivationFunctionType.Identity,
                                     bias=smallw["b_o2"][:, :])
                nc.sync.dma_start(out=outp[:, off:off + seg], in_=ot[:, 0:seg])
                off += seg

    nc.compile()
    return nc


# ---------------------------------------------------------------------------
def prep(inputs: dict, cfg: Cfg):
    import ml_dtypes
    bf = ml_dtypes.bfloat16
    N = cfg.n_real
    src = np.asarray(inputs["edge_index"][0]).astype(np.int64)
    dst = np.asarray(inputs["edge_index"][1]).astype(np.int64)
    loop = np.arange(N, dtype=np.int64)
    src = np.concatenate([src, loop]); dst = np.concatenate([dst, loop])
    E = src.shape[0]

    nbins = cfg.ncores * cfg.tpc
    deg = np.bincount(dst, minlength=cfg.npad).astype(np.int64)
    order = np.argsort(-deg, kind="stable")
    binof = np.empty(cfg.npad, dtype=np.int64)
    slotof = np.empty(cfg.npad, dtype=np.int64)
    pos = np.arange(cfg.npad)
    binof[order] = pos % nbins
    slotof[order] = pos // nbins
    new_id = binof * P + slotof
    bin_load = np.bincount(binof[dst], minlength=nbins)
    assert bin_load.max() <= cfg.mb * P, (
        f"bin overflow: {bin_load.max()} > {cfg.mb * P}; raise cfg.mb")

    ebin = binof[dst]
    eorder = np.argsort(ebin, kind="stable")
    ebin_s = ebin[eorder]
    src_s = src[eorder]; dst_s = dst[eorder]
    starts = np.zeros(nbins + 1, dtype=np.int64)
    np.cumsum(np.bincount(ebin_s, minlength=nbins), out=starts[1:])
    eoff = np.arange(E) - starts[ebin_s]

    B = ebin_s * cfg.mb + eoff // P
    pp_ = (eoff % P).astype(np.int64)
    core = ebin_s // cfg.tpc
    Bc = B - core * cfg.nbatch

    idx_src = np.zeros((cfg.ncores, P, cfg.nbatch), dtype=np.int32)
    idx_ald = np.zeros((cfg.ncores, P, cfg.nbatch), dtype=np.int32)
    dstloc = np.full((cfg.ncores, P, cfg.nbatch), 200.0, dtype=bf)
    idx_src[core, pp_, Bc] = new_id[src_s].astype(np.int32)
    idx_ald[core, pp_, Bc] = (new_id[dst_s] % cfg.npc).astype(np.int32)
    dstloc[core, pp_, Bc] = (new_id[dst_s] % P).astype(np.float32).astype(bf)

    inv = np.empty(cfg.npad, dtype=np.int64)
    inv[new_id] = np.arange(cfg.npad)

    desT = np.ascontiguousarray(np.asarray(inputs["des"], np.float32).T)
    twT = np.ascontiguousarray(np.asarray(inputs["tweet"], np.float32).T)
    npT = np.ascontiguousarray(np.asarray(inputs["num_prop"], np.float32).T)
    cpT = np.ascontiguousarray(np.asarray(inputs["cat_prop"], np.float32).T)

    def slab(mT, c):
        cols = inv[c * cfg.npc:(c + 1) * cfg.npc]
        out = np.zeros((mT.shape[0], cfg.npc), dtype=np.float32)
        real = cols < N
        out[:, real] = mT[:, cols[real]]
        return out

    A_src1 = np.zeros((128, 4), np.float32); A_dst1 = np.zeros((128, 4), np.float32)
    a_s1 = np.asarray(inputs["gat1_asrc"], np.float32)
    a_d1 = np.asarray(inputs["gat1_adst"], np.float32)
    for h in range(4):
        A_src1[h * 32:(h + 1) * 32, h] = a_s1[h]
        A_dst1[h * 32:(h + 1) * 32, h] = a_d1[h]
    A_src2 = np.asarray(inputs["gat2_asrc"], np.float32).reshape(128, 1)
    A_dst2 = np.asarray(inputs["gat2_adst"], np.float32).reshape(128, 1)

    b_enc = np.concatenate([np.asarray(inputs[k], np.float32) for k in
                            ("b_des", "b_tw", "b_np", "b_cp")]).reshape(128, 1)
    iota_bf = np.tile(np.arange(P, dtype=np.float32), (P, 1)).astype(bf)
    ident = np.eye(P, dtype=np.float32)

    in_maps = []
    for c in range(cfg.ncores):
        m = {
            "desT": slab(desT, c), "tweetT": slab(twT, c),
            "numT": slab(npT, c), "catT": slab(cpT, c),
            "W_des": np.asarray(inputs["W_des"], np.float32),
            "W_tw": np.asarray(inputs["W_tw"], np.float32),
            "W_np": np.asarray(inputs["W_np"], np.float32),
            "W_cp": np.asarray(inputs["W_cp"], np.float32),
            "b_enc": b_enc,
            "W_in": np.asarray(inputs["W_in"], np.float32),
            "b_in": np.asarray(inputs["b_in"], np.float32).reshape(128, 1),
            "gat1_W": np.asarray(inputs["gat1_W"], np.float32),
            "A_src1": A_src1, "A_dst1": A_dst1,
            "b1": np.asarray(inputs["gat1_b"], np.float32).reshape(128, 1),
            "gat2_W": np.asarray(inputs["gat2_W"], np.float32),
            "A_src2": A_src2, "A_dst2": A_dst2,
            "b2": np.asarray(inputs["gat2_b"], np.float32).reshape(128, 1),
            "W_o1": np.asarray(inputs["W_o1"], np.float32),
            "b_o1": np.asarray(inputs["b_o1"], np.float32).reshape(128, 1),
            "W_o2": np.asarray(inputs["W_o2"], np.float32),
            "b_o2": np.asarray(inputs["b_o2"], np.float32).reshape(2, 1),
            "iota_bf": iota_bf, "ident": ident, "ident_bf": ident.astype(bf),
            "idx_src": idx_src[c], "dstloc": dstloc[c],
        }
        in_maps.append(m)
    return in_maps, {"new_id": new_id, "N": N}


def assemble(results, meta, cfg: Cfg):
    outT = np.concatenate([np.asarray(r["outT"]) for r in results], axis=1)
    return outT.T[meta["new_id"][:meta["N"]]].astype(np.float32)


_CACHE = {}

def _run(inputs, trace=False):
    cfg = FULL
    if "nc" not in _CACHE:
        _CACHE["nc"] = build(cfg)
    from concourse.bass_utils import run_bass_kernel_spmd
    in_maps, meta = prep(inputs, cfg)
    res = run_bass_kernel_spmd(_CACHE["nc"], in_maps,
                               core_ids=list(range(cfg.ncores)), trace=trace)
    return assemble(res.results, meta, cfg), res


def kernel(**inputs):
    out, _ = _run(inputs)
    return out

